# revision 1
# baseline (speedup 1.0000x reference)
"""DGMNet forward kernel v2 for Trainium2, 8-core data parallel.

Same dataflow as v1 (feature-major activations, bias rows folded into
the matmuls, xt projections recomputed per step as PSUM-accumulating
matmul pairs) but in bf16 end-to-end:

- xt / w1 / w2 / wf live in DRAM+SBUF as bf16 (halves HBM traffic and
  makes every DVE tensor_tensor run in 2x_1P packed mode).
- tanh outputs, S, SR, and all elementwise intermediates are bf16;
  PSUM accumulation stays fp32.
- Engine placement tuned against the timeline cost model:
  Act = wide tanh per step ([128, 3*nt] in one instruction),
  Pool = S*R (scalar_tensor_tensor form),
  DVE = ZS, T1=(G-1)*H (PSUM read), Snew, and the output copy.
"""

import warnings

warnings.filterwarnings("ignore")

import numpy as np

B = 262144
XD = 100
HID = 128
N_CORES = 8
BS = B // N_CORES  # 32768 rows per core
KXT = XD + 2  # x features + t row + ones row
NT = 512  # batch columns per tile


def _build(n_steps: int, bs: int, nt: int, reps: int = 1, hw_loop: int = 0,
           out_copy_engine: str = "vector"):
    import concourse.bacc as bacc
    import concourse.tile as tile
    import concourse.mybir as mybir
    from contextlib import ExitStack, nullcontext

    f32 = mybir.dt.float32
    bf16 = mybir.dt.bfloat16
    Tanh = mybir.ActivationFunctionType.Tanh
    sub = mybir.AluOpType.subtract
    mult = mybir.AluOpType.mult

    ntiles = bs // nt
    nc = bacc.Bacc("TRN2", target_bir_lowering=False, debug=False,
                   num_devices=N_CORES)

    xt = nc.dram_tensor("xt", [KXT, bs], bf16, kind="ExternalInput").ap()
    w1 = nc.dram_tensor("w1", [KXT, 5 * HID], bf16, kind="ExternalInput").ap()
    w2 = nc.dram_tensor("w2", [HID, 4 * HID], bf16, kind="ExternalInput").ap()
    wf = nc.dram_tensor("wf", [HID, 1], bf16, kind="ExternalInput").ap()
    out = nc.dram_tensor("out", [1, bs], f32, kind="ExternalOutput").ap()

    with tile.TileContext(nc) as tc:
        with ExitStack() as ctx:
            consts = ctx.enter_context(tc.tile_pool(name="consts", bufs=1))
            xpool = ctx.enter_context(tc.tile_pool(name="xp", bufs=7))
            spool = ctx.enter_context(tc.tile_pool(name="sp", bufs=12))
            zpool = ctx.enter_context(tc.tile_pool(name="zp", bufs=4))
            epool = ctx.enter_context(tc.tile_pool(name="ep", bufs=6))
            psum = ctx.enter_context(
                tc.tile_pool(name="ps", bufs=2, space="PSUM"))

            w1_t = consts.tile([KXT, 5 * HID], bf16)
            nc.sync.dma_start(w1_t[:], w1[:, :])
            w2_t = consts.tile([HID, 4 * HID], bf16)
            nc.sync.dma_start(w2_t[:], w2[:, :])
            wf_t = consts.tile([HID, 1], bf16)
            nc.sync.dma_start(wf_t[:], wf[:, :])

            def w1s(k):
                return w1_t[:, k * HID:(k + 1) * HID]

            def w2s(k):
                return w2_t[:, k * HID:(k + 1) * HID]

            # Optional HW loop repeating the whole pass (timing rig only)
            loop_cm = (tc.For_i(0, hw_loop, 1,
                                hint_engines=(mybir.EngineType.PE,
                                              mybir.EngineType.Activation,
                                              mybir.EngineType.DVE,
                                              mybir.EngineType.SP,
                                              mybir.EngineType.Pool))
                       if hw_loop else nullcontext())
            ctx.enter_context(loop_cm)

            GRP = 6  # tiles software-pipelined per step loop
            for j0 in range(0, ntiles * reps, GRP):
                js = [jj % ntiles for jj in
                      range(j0, min(j0 + GRP, ntiles * reps))]
                xr = {}
                S = {}
                for j in js:
                    x_t = xpool.tile([KXT, nt], bf16, tag="x", name="x_t")
                    nc.sync.dma_start(x_t[:], xt[:, j * nt:(j + 1) * nt])
                    xr[j] = x_t[:]
                # S1 per tile through the 1-bank r-ring
                for j in js:
                    ps1 = psum.tile([HID, nt], f32, tag="r", name="ps1")
                    nc.tensor.matmul(ps1[:], w1s(0), xr[j],
                                     start=True, stop=True)
                    S1 = spool.tile([HID, nt], bf16, tag="S", name="S1")
                    nc.scalar.activation(S1[:], ps1[:], Tanh)
                    S[j] = S1

                for _ in range(n_steps):
                    for j in js:
                        Sr = S[j][:]
                        # R first: it unblocks the SR -> H chain
                        pr = psum.tile([HID, nt], f32, tag="r", name="pr")
                        nc.tensor.matmul(pr[:], w1s(3), xr[j],
                                         start=True, stop=False)
                        nc.tensor.matmul(pr[:], w2s(2), Sr,
                                         start=False, stop=True)
                        Rt = zpool.tile([HID, nt], bf16, tag="R", name="Rt")
                        nc.scalar.activation(Rt[:], pr[:], Tanh)
                        SR = epool.tile([HID, nt], bf16, tag="SR", name="SR")
                        nc.gpsimd.tensor_mul(SR[:], S[j][:], Rt[:])

                        pzg = psum.tile([HID, 2 * nt], f32, tag="zg",
                                        name="pzg")
                        for k in range(2):
                            sl = pzg[:, k * nt:(k + 1) * nt]
                            nc.tensor.matmul(sl, w1s(1 + k), xr[j],
                                             start=True, stop=False)
                            nc.tensor.matmul(sl, w2s(k), Sr,
                                             start=False, stop=True)
                        ZG = zpool.tile([HID, 2 * nt], bf16, tag="ZG",
                                        name="ZG")
                        nc.scalar.activation(ZG[:], pzg[:], Tanh)
                        Z = ZG[:, 0:nt]
                        G = ZG[:, nt:2 * nt]

                        ph = psum.tile([HID, nt], f32, tag="h", name="ph")
                        nc.tensor.matmul(ph[:], w1s(4), xr[j],
                                         start=True, stop=False)
                        nc.tensor.matmul(ph[:], w2s(3), SR[:],
                                         start=False, stop=True)

                        ZS = epool.tile([HID, nt], bf16, tag="ZS", name="ZS")
                        nc.vector.tensor_mul(ZS[:], Z, S[j][:])
                        T1 = epool.tile([HID, nt], bf16, tag="T1", name="T1")
                        nc.vector.scalar_tensor_tensor(T1[:], G, 1.0, ph[:],
                                                       op0=sub, op1=mult)
                        Snew = spool.tile([HID, nt], bf16, tag="S",
                                          name="Snew")
                        nc.vector.tensor_sub(Snew[:], ZS[:], T1[:])
                        S[j] = Snew

                for j in js:
                    po = psum.tile([1, nt], f32, tag="h", name="po")
                    nc.tensor.matmul(po[:], wf_t[:],
                                     S[j][:],
                                     start=True, stop=True)
                    o_t = xpool.tile([1, nt], f32, tag="o", name="o_t")
                    if out_copy_engine == "scalar":
                        nc.scalar.copy(o_t[:], po[:])
                    else:
                        nc.vector.tensor_copy(o_t[:], po[:])
                    nc.sync.dma_start(out[:, j * nt:(j + 1) * nt], o_t[:])

    nc.compile()
    return nc


_cache = {}


def _get_nc(n_steps: int, bs: int = BS, nt: int = NT):
    key = (n_steps, bs, nt)
    if key not in _cache:
        _cache[key] = _build(n_steps, bs, nt)
    return _cache[key]


def _pack_host(x, t, Sw_w, Sw_b, Uz_w, Uz_b, Wsz_w, Wsz_b, Ug_w, Ug_b, Wsg_w,
               Wsg_b, Ur_w, Ur_b, Wsr_w, Wsr_b, Uh_w, Uh_b, Wsh_w, Wsh_b,
               Wf_w):
    import ml_dtypes

    bf16 = ml_dtypes.bfloat16
    f32 = np.float32
    b_total = x.shape[0]
    xt_full = np.empty((KXT, b_total), dtype=bf16)
    xt_full[:XD, :] = np.asarray(x, dtype=f32).T.astype(bf16)
    xt_full[XD, :] = np.asarray(t, dtype=f32)[:, 0].astype(bf16)
    xt_full[XD + 1, :] = 1.0

    def blk(w, b):
        # [101, 128] weights + folded bias row -> [102, 128]
        return np.concatenate(
            [np.asarray(w, f32),
             np.asarray(b, f32).reshape(1, HID)], axis=0)

    w1 = np.concatenate([
        blk(Sw_w, Sw_b),
        blk(Uz_w, np.asarray(Uz_b) + np.asarray(Wsz_b)),
        blk(Ug_w, np.asarray(Ug_b) + np.asarray(Wsg_b)),
        blk(Ur_w, np.asarray(Ur_b) + np.asarray(Wsr_b)),
        blk(Uh_w, np.asarray(Uh_b) + np.asarray(Wsh_b)),
    ], axis=1).astype(bf16)
    w2 = np.concatenate([Wsz_w, Wsg_w, Wsr_w, Wsh_w], axis=1).astype(bf16)
    wf = np.asarray(Wf_w, dtype=f32).reshape(HID, 1).astype(bf16)
    return xt_full, w1, w2, wf


def kernel(x, t, Sw_w, Sw_b, Uz_w, Uz_b, Wsz_w, Wsz_b, Ug_w, Ug_b, Wsg_w,
           Wsg_b, Ur_w, Ur_b, Wsr_w, Wsr_b, Uh_w, Uh_b, Wsh_w, Wsh_b, Wf_w,
           Wf_b, n_layers):
    from concourse.bass_utils import run_bass_kernel_spmd

    x = np.asarray(x)
    t = np.asarray(t)
    b_total = x.shape[0]
    assert b_total % N_CORES == 0
    bs = b_total // N_CORES
    n_steps = int(n_layers) - 1

    xt_full, w1, w2, wf = _pack_host(
        x, t, Sw_w, Sw_b, Uz_w, Uz_b, Wsz_w, Wsz_b, Ug_w, Ug_b, Wsg_w, Wsg_b,
        Ur_w, Ur_b, Wsr_w, Wsr_b, Uh_w, Uh_b, Wsh_w, Wsh_b, Wf_w)

    nc = _get_nc(n_steps, bs)

    in_maps = []
    for c in range(N_CORES):
        in_maps.append({
            "xt": np.ascontiguousarray(xt_full[:, c * bs:(c + 1) * bs]),
            "w1": w1,
            "w2": w2,
            "wf": wf,
        })

    res = None
    for attempt in range(3):
        try:
            res = run_bass_kernel_spmd(nc, in_maps,
                                       core_ids=list(range(N_CORES)))
            break
        except Exception:
            if attempt == 2:
                raise
            import time as _time
            _time.sleep(5.0)
    out = np.empty((b_total, 1), dtype=np.float32)
    bf = np.float32(np.asarray(Wf_b).reshape(-1)[0])
    for c in range(N_CORES):
        out[c * bs:(c + 1) * bs, 0] = res.results[c]["out"][0] + bf
    return out



# revision 2
# speedup vs baseline: 1.0785x; 1.0785x over previous
"""DGMNet forward kernel v2 for Trainium2, 8-core data parallel.

Same dataflow as v1 (feature-major activations, bias rows folded into
the matmuls, xt projections recomputed per step as PSUM-accumulating
matmul pairs) but in bf16 end-to-end:

- xt / w1 / w2 / wf live in DRAM+SBUF as bf16 (halves HBM traffic and
  makes every DVE tensor_tensor run in 2x_1P packed mode).
- tanh outputs, S, SR, and all elementwise intermediates are bf16;
  PSUM accumulation stays fp32.
- Engine placement tuned against the timeline cost model and HW
  measurement: ALL elementwise on the DVE (SR, ZS, T1, Snew, output
  copy); the GPSIMD/Pool engine is left idle on purpose -- it shares
  an SBUF port with the DVE on real HW, and moving S*R off it
  measured 488 -> 448 us even though per-op it looked free.
"""

import warnings

warnings.filterwarnings("ignore")

import numpy as np

B = 262144
XD = 100
HID = 128
N_CORES = 8
BS = B // N_CORES  # 32768 rows per core
KXT = XD + 2  # x features + t row + ones row
NT = 512  # batch columns per tile


def _build(n_steps: int, bs: int, nt: int, reps: int = 1, hw_loop: int = 0,
           out_copy_engine: str = "vector"):
    import concourse.bacc as bacc
    import concourse.tile as tile
    import concourse.mybir as mybir
    from contextlib import ExitStack, nullcontext

    f32 = mybir.dt.float32
    bf16 = mybir.dt.bfloat16
    Tanh = mybir.ActivationFunctionType.Tanh
    sub = mybir.AluOpType.subtract
    mult = mybir.AluOpType.mult

    ntiles = bs // nt
    nc = bacc.Bacc("TRN2", target_bir_lowering=False, debug=False,
                   num_devices=N_CORES)

    xt = nc.dram_tensor("xt", [KXT, bs], bf16, kind="ExternalInput").ap()
    w1 = nc.dram_tensor("w1", [KXT, 5 * HID], bf16, kind="ExternalInput").ap()
    w2 = nc.dram_tensor("w2", [HID, 4 * HID], bf16, kind="ExternalInput").ap()
    wf = nc.dram_tensor("wf", [HID, 1], bf16, kind="ExternalInput").ap()
    out = nc.dram_tensor("out", [1, bs], f32, kind="ExternalOutput").ap()

    with tile.TileContext(nc) as tc:
        with ExitStack() as ctx:
            consts = ctx.enter_context(tc.tile_pool(name="consts", bufs=1))
            xpool = ctx.enter_context(tc.tile_pool(name="xp", bufs=7))
            spool = ctx.enter_context(tc.tile_pool(name="sp", bufs=12))
            zpool = ctx.enter_context(tc.tile_pool(name="zp", bufs=4))
            epool = ctx.enter_context(tc.tile_pool(name="ep", bufs=6))
            psum = ctx.enter_context(
                tc.tile_pool(name="ps", bufs=2, space="PSUM"))

            w1_t = consts.tile([KXT, 5 * HID], bf16)
            nc.sync.dma_start(w1_t[:], w1[:, :])
            w2_t = consts.tile([HID, 4 * HID], bf16)
            nc.sync.dma_start(w2_t[:], w2[:, :])
            wf_t = consts.tile([HID, 1], bf16)
            nc.sync.dma_start(wf_t[:], wf[:, :])

            def w1s(k):
                return w1_t[:, k * HID:(k + 1) * HID]

            def w2s(k):
                return w2_t[:, k * HID:(k + 1) * HID]

            # Optional HW loop repeating the whole pass (timing rig only)
            loop_cm = (tc.For_i(0, hw_loop, 1,
                                hint_engines=(mybir.EngineType.PE,
                                              mybir.EngineType.Activation,
                                              mybir.EngineType.DVE,
                                              mybir.EngineType.SP,
                                              mybir.EngineType.Pool))
                       if hw_loop else nullcontext())
            ctx.enter_context(loop_cm)

            GRP = 6  # tiles software-pipelined per step loop
            for j0 in range(0, ntiles * reps, GRP):
                js = [jj % ntiles for jj in
                      range(j0, min(j0 + GRP, ntiles * reps))]
                xr = {}
                S = {}
                for j in js:
                    x_t = xpool.tile([KXT, nt], bf16, tag="x", name="x_t")
                    nc.sync.dma_start(x_t[:], xt[:, j * nt:(j + 1) * nt])
                    xr[j] = x_t[:]
                # S1 per tile through the 1-bank r-ring
                for j in js:
                    ps1 = psum.tile([HID, nt], f32, tag="r", name="ps1")
                    nc.tensor.matmul(ps1[:], w1s(0), xr[j],
                                     start=True, stop=True)
                    S1 = spool.tile([HID, nt], bf16, tag="S", name="S1")
                    nc.scalar.activation(S1[:], ps1[:], Tanh)
                    S[j] = S1

                for _ in range(n_steps):
                    for j in js:
                        Sr = S[j][:]
                        # R first: it unblocks the SR -> H chain
                        pr = psum.tile([HID, nt], f32, tag="r", name="pr")
                        nc.tensor.matmul(pr[:], w1s(3), xr[j],
                                         start=True, stop=False)
                        nc.tensor.matmul(pr[:], w2s(2), Sr,
                                         start=False, stop=True)
                        Rt = zpool.tile([HID, nt], bf16, tag="R", name="Rt")
                        nc.scalar.activation(Rt[:], pr[:], Tanh)
                        SR = epool.tile([HID, nt], bf16, tag="SR", name="SR")
                        nc.vector.tensor_mul(SR[:], S[j][:], Rt[:])

                        pzg = psum.tile([HID, 2 * nt], f32, tag="zg",
                                        name="pzg")
                        for k in range(2):
                            sl = pzg[:, k * nt:(k + 1) * nt]
                            nc.tensor.matmul(sl, w1s(1 + k), xr[j],
                                             start=True, stop=False)
                            nc.tensor.matmul(sl, w2s(k), Sr,
                                             start=False, stop=True)
                        ZG = zpool.tile([HID, 2 * nt], bf16, tag="ZG",
                                        name="ZG")
                        nc.scalar.activation(ZG[:], pzg[:], Tanh)
                        Z = ZG[:, 0:nt]
                        G = ZG[:, nt:2 * nt]

                        ph = psum.tile([HID, nt], f32, tag="h", name="ph")
                        nc.tensor.matmul(ph[:], w1s(4), xr[j],
                                         start=True, stop=False)
                        nc.tensor.matmul(ph[:], w2s(3), SR[:],
                                         start=False, stop=True)

                        ZS = epool.tile([HID, nt], bf16, tag="ZS", name="ZS")
                        nc.vector.tensor_mul(ZS[:], Z, S[j][:])
                        T1 = epool.tile([HID, nt], bf16, tag="T1", name="T1")
                        nc.vector.scalar_tensor_tensor(T1[:], G, 1.0, ph[:],
                                                       op0=sub, op1=mult)
                        Snew = spool.tile([HID, nt], bf16, tag="S",
                                          name="Snew")
                        nc.vector.tensor_sub(Snew[:], ZS[:], T1[:])
                        S[j] = Snew

                for j in js:
                    po = psum.tile([1, nt], f32, tag="h", name="po")
                    nc.tensor.matmul(po[:], wf_t[:],
                                     S[j][:],
                                     start=True, stop=True)
                    o_t = xpool.tile([1, nt], f32, tag="o", name="o_t")
                    if out_copy_engine == "scalar":
                        nc.scalar.copy(o_t[:], po[:])
                    else:
                        nc.vector.tensor_copy(o_t[:], po[:])
                    nc.sync.dma_start(out[:, j * nt:(j + 1) * nt], o_t[:])

    nc.compile()
    return nc


_cache = {}


def _get_nc(n_steps: int, bs: int = BS, nt: int = NT):
    key = (n_steps, bs, nt)
    if key not in _cache:
        _cache[key] = _build(n_steps, bs, nt)
    return _cache[key]


def _pack_host(x, t, Sw_w, Sw_b, Uz_w, Uz_b, Wsz_w, Wsz_b, Ug_w, Ug_b, Wsg_w,
               Wsg_b, Ur_w, Ur_b, Wsr_w, Wsr_b, Uh_w, Uh_b, Wsh_w, Wsh_b,
               Wf_w):
    import ml_dtypes

    bf16 = ml_dtypes.bfloat16
    f32 = np.float32
    b_total = x.shape[0]
    xt_full = np.empty((KXT, b_total), dtype=bf16)
    xt_full[:XD, :] = np.asarray(x, dtype=f32).T.astype(bf16)
    xt_full[XD, :] = np.asarray(t, dtype=f32)[:, 0].astype(bf16)
    xt_full[XD + 1, :] = 1.0

    def blk(w, b):
        # [101, 128] weights + folded bias row -> [102, 128]
        return np.concatenate(
            [np.asarray(w, f32),
             np.asarray(b, f32).reshape(1, HID)], axis=0)

    w1 = np.concatenate([
        blk(Sw_w, Sw_b),
        blk(Uz_w, np.asarray(Uz_b) + np.asarray(Wsz_b)),
        blk(Ug_w, np.asarray(Ug_b) + np.asarray(Wsg_b)),
        blk(Ur_w, np.asarray(Ur_b) + np.asarray(Wsr_b)),
        blk(Uh_w, np.asarray(Uh_b) + np.asarray(Wsh_b)),
    ], axis=1).astype(bf16)
    w2 = np.concatenate([Wsz_w, Wsg_w, Wsr_w, Wsh_w], axis=1).astype(bf16)
    wf = np.asarray(Wf_w, dtype=f32).reshape(HID, 1).astype(bf16)
    return xt_full, w1, w2, wf


def kernel(x, t, Sw_w, Sw_b, Uz_w, Uz_b, Wsz_w, Wsz_b, Ug_w, Ug_b, Wsg_w,
           Wsg_b, Ur_w, Ur_b, Wsr_w, Wsr_b, Uh_w, Uh_b, Wsh_w, Wsh_b, Wf_w,
           Wf_b, n_layers):
    from concourse.bass_utils import run_bass_kernel_spmd

    x = np.asarray(x)
    t = np.asarray(t)
    b_total = x.shape[0]
    assert b_total % N_CORES == 0
    bs = b_total // N_CORES
    n_steps = int(n_layers) - 1

    xt_full, w1, w2, wf = _pack_host(
        x, t, Sw_w, Sw_b, Uz_w, Uz_b, Wsz_w, Wsz_b, Ug_w, Ug_b, Wsg_w, Wsg_b,
        Ur_w, Ur_b, Wsr_w, Wsr_b, Uh_w, Uh_b, Wsh_w, Wsh_b, Wf_w)

    nc = _get_nc(n_steps, bs)

    in_maps = []
    for c in range(N_CORES):
        in_maps.append({
            "xt": np.ascontiguousarray(xt_full[:, c * bs:(c + 1) * bs]),
            "w1": w1,
            "w2": w2,
            "wf": wf,
        })

    res = None
    for attempt in range(3):
        try:
            res = run_bass_kernel_spmd(nc, in_maps,
                                       core_ids=list(range(N_CORES)))
            break
        except Exception:
            if attempt == 2:
                raise
            import time as _time
            _time.sleep(5.0)
    out = np.empty((b_total, 1), dtype=np.float32)
    bf = np.float32(np.asarray(Wf_b).reshape(-1)[0])
    for c in range(N_CORES):
        out[c * bs:(c + 1) * bs, 0] = res.results[c]["out"][0] + bf
    return out



# revision 3
# speedup vs baseline: 1.1098x; 1.0290x over previous
"""DGMNet forward kernel v2 for Trainium2, 8-core data parallel.

Same dataflow as v1 (feature-major activations, bias rows folded into
the matmuls, xt projections recomputed per step as PSUM-accumulating
matmul pairs) but in bf16 end-to-end:

- xt / w1 / w2 / wf live in DRAM+SBUF as bf16 (halves HBM traffic and
  makes every DVE tensor_tensor run in 2x_1P packed mode).
- tanh outputs, S, SR, and all elementwise intermediates are bf16;
  PSUM accumulation stays fp32.
- Engine placement tuned against the timeline cost model and HW
  measurement: ALL elementwise on the DVE (SR, ZS, T1, Snew, output
  copy); the GPSIMD/Pool engine is left idle on purpose -- it shares
  an SBUF port with the DVE on real HW, and moving S*R off it
  measured 488 -> 448 us even though per-op it looked free.
- Final projections are col-tiled (tile_position): 3 tiles' output
  rows land at partitions 0/32/64 of one PSUM bank, so one DVE copy
  and bank evacuate 3 tiles (the copy cost is free-dim driven), and
  the 3 matmuls run concurrently in the PE array's column groups.
"""

import warnings

warnings.filterwarnings("ignore")

import numpy as np

B = 262144
XD = 100
HID = 128
N_CORES = 8
BS = B // N_CORES  # 32768 rows per core
KXT = XD + 2  # x features + t row + ones row
NT = 512  # batch columns per tile


def _build(n_steps: int, bs: int, nt: int, reps: int = 1, hw_loop: int = 0,
           out_copy_engine: str = "vector"):
    import concourse.bacc as bacc
    import concourse.tile as tile
    import concourse.mybir as mybir
    from contextlib import ExitStack, nullcontext

    f32 = mybir.dt.float32
    bf16 = mybir.dt.bfloat16
    Tanh = mybir.ActivationFunctionType.Tanh
    sub = mybir.AluOpType.subtract
    mult = mybir.AluOpType.mult

    ntiles = bs // nt
    nc = bacc.Bacc("TRN2", target_bir_lowering=False, debug=False,
                   num_devices=N_CORES)

    xt = nc.dram_tensor("xt", [KXT, bs], bf16, kind="ExternalInput").ap()
    w1 = nc.dram_tensor("w1", [KXT, 5 * HID], bf16, kind="ExternalInput").ap()
    w2 = nc.dram_tensor("w2", [HID, 4 * HID], bf16, kind="ExternalInput").ap()
    wf = nc.dram_tensor("wf", [HID, 1], bf16, kind="ExternalInput").ap()
    out = nc.dram_tensor("out", [1, bs], f32, kind="ExternalOutput").ap()

    with tile.TileContext(nc) as tc:
        with ExitStack() as ctx:
            consts = ctx.enter_context(tc.tile_pool(name="consts", bufs=1))
            xpool = ctx.enter_context(tc.tile_pool(name="xp", bufs=7))
            spool = ctx.enter_context(tc.tile_pool(name="sp", bufs=12))
            zpool = ctx.enter_context(tc.tile_pool(name="zp", bufs=4))
            epool = ctx.enter_context(tc.tile_pool(name="ep", bufs=6))
            psum = ctx.enter_context(
                tc.tile_pool(name="ps", bufs=2, space="PSUM"))

            w1_t = consts.tile([KXT, 5 * HID], bf16)
            nc.sync.dma_start(w1_t[:], w1[:, :])
            w2_t = consts.tile([HID, 4 * HID], bf16)
            nc.sync.dma_start(w2_t[:], w2[:, :])
            wf_t = consts.tile([HID, 1], bf16)
            nc.sync.dma_start(wf_t[:], wf[:, :])

            def w1s(k):
                return w1_t[:, k * HID:(k + 1) * HID]

            def w2s(k):
                return w2_t[:, k * HID:(k + 1) * HID]

            # Optional HW loop repeating the whole pass (timing rig only)
            loop_cm = (tc.For_i(0, hw_loop, 1,
                                hint_engines=(mybir.EngineType.PE,
                                              mybir.EngineType.Activation,
                                              mybir.EngineType.DVE,
                                              mybir.EngineType.SP,
                                              mybir.EngineType.Pool))
                       if hw_loop else nullcontext())
            ctx.enter_context(loop_cm)

            GRP = 6  # tiles software-pipelined per step loop
            for j0 in range(0, ntiles * reps, GRP):
                js = [jj % ntiles for jj in
                      range(j0, min(j0 + GRP, ntiles * reps))]
                xr = {}
                S = {}
                for j in js:
                    x_t = xpool.tile([KXT, nt], bf16, tag="x", name="x_t")
                    nc.sync.dma_start(x_t[:], xt[:, j * nt:(j + 1) * nt])
                    xr[j] = x_t[:]
                # S1 per tile, alternating r/h rings (the h ring has
                # slack now that the outputs batch 3-per-bank)
                for i, j in enumerate(js):
                    ps1 = psum.tile([HID, nt], f32,
                                    tag=("r" if i % 2 == 0 else "h"),
                                    name="ps1")
                    nc.tensor.matmul(ps1[:], w1s(0), xr[j],
                                     start=True, stop=True)
                    S1 = spool.tile([HID, nt], bf16, tag="S", name="S1")
                    nc.scalar.activation(S1[:], ps1[:], Tanh)
                    S[j] = S1

                for _ in range(n_steps):
                    for j in js:
                        Sr = S[j][:]
                        # R first: it unblocks the SR -> H chain
                        pr = psum.tile([HID, nt], f32, tag="r", name="pr")
                        nc.tensor.matmul(pr[:], w1s(3), xr[j],
                                         start=True, stop=False)
                        nc.tensor.matmul(pr[:], w2s(2), Sr,
                                         start=False, stop=True)
                        Rt = zpool.tile([HID, nt], bf16, tag="R", name="Rt")
                        nc.scalar.activation(Rt[:], pr[:], Tanh)
                        SR = epool.tile([HID, nt], bf16, tag="SR", name="SR")
                        nc.vector.tensor_mul(SR[:], S[j][:], Rt[:])

                        pzg = psum.tile([HID, 2 * nt], f32, tag="zg",
                                        name="pzg")
                        for k in range(2):
                            sl = pzg[:, k * nt:(k + 1) * nt]
                            nc.tensor.matmul(sl, w1s(1 + k), xr[j],
                                             start=True, stop=False)
                            nc.tensor.matmul(sl, w2s(k), Sr,
                                             start=False, stop=True)
                        ZG = zpool.tile([HID, 2 * nt], bf16, tag="ZG",
                                        name="ZG")
                        nc.scalar.activation(ZG[:], pzg[:], Tanh)
                        Z = ZG[:, 0:nt]
                        G = ZG[:, nt:2 * nt]

                        ph = psum.tile([HID, nt], f32, tag="h", name="ph")
                        nc.tensor.matmul(ph[:], w1s(4), xr[j],
                                         start=True, stop=False)
                        nc.tensor.matmul(ph[:], w2s(3), SR[:],
                                         start=False, stop=True)

                        ZS = epool.tile([HID, nt], bf16, tag="ZS", name="ZS")
                        nc.vector.tensor_mul(ZS[:], Z, S[j][:])
                        T1 = epool.tile([HID, nt], bf16, tag="T1", name="T1")
                        nc.vector.scalar_tensor_tensor(T1[:], G, 1.0, ph[:],
                                                       op0=sub, op1=mult)
                        Snew = spool.tile([HID, nt], bf16, tag="S",
                                          name="Snew")
                        nc.vector.tensor_sub(Snew[:], ZS[:], T1[:])
                        S[j] = Snew

                # Final projections, 3 tiles per PSUM bank: col-tiled
                # matmuls land each tile's output row at partition 0/32/64
                # of ONE bank, so a single DVE copy (cost is free-dim
                # driven, partition-count free) evacuates up to 3 tiles.
                for c0 in range(0, len(js), 3):
                    chunk = js[c0:c0 + 3]
                    po = psum.tile([HID, nt], f32, tag="h", name="po")
                    for i, j in enumerate(chunk):
                        nc.tensor.matmul(po[32 * i:32 * i + 1, :], wf_t[:],
                                         S[j][:], start=True, stop=True,
                                         tile_position=(0, 32 * i))
                    o_t = xpool.tile([65, nt], f32, tag="o", name="o_t")
                    nc.vector.tensor_copy(o_t[:], po[0:65, :])
                    for i, j in enumerate(chunk):
                        nc.sync.dma_start(out[:, j * nt:(j + 1) * nt],
                                          o_t[32 * i:32 * i + 1, :])

    nc.compile()
    return nc


_cache = {}


def _get_nc(n_steps: int, bs: int = BS, nt: int = NT):
    key = (n_steps, bs, nt)
    if key not in _cache:
        _cache[key] = _build(n_steps, bs, nt)
    return _cache[key]


def _pack_host(x, t, Sw_w, Sw_b, Uz_w, Uz_b, Wsz_w, Wsz_b, Ug_w, Ug_b, Wsg_w,
               Wsg_b, Ur_w, Ur_b, Wsr_w, Wsr_b, Uh_w, Uh_b, Wsh_w, Wsh_b,
               Wf_w):
    import ml_dtypes

    bf16 = ml_dtypes.bfloat16
    f32 = np.float32
    b_total = x.shape[0]
    xt_full = np.empty((KXT, b_total), dtype=bf16)
    xt_full[:XD, :] = np.asarray(x, dtype=f32).T.astype(bf16)
    xt_full[XD, :] = np.asarray(t, dtype=f32)[:, 0].astype(bf16)
    xt_full[XD + 1, :] = 1.0

    def blk(w, b):
        # [101, 128] weights + folded bias row -> [102, 128]
        return np.concatenate(
            [np.asarray(w, f32),
             np.asarray(b, f32).reshape(1, HID)], axis=0)

    w1 = np.concatenate([
        blk(Sw_w, Sw_b),
        blk(Uz_w, np.asarray(Uz_b) + np.asarray(Wsz_b)),
        blk(Ug_w, np.asarray(Ug_b) + np.asarray(Wsg_b)),
        blk(Ur_w, np.asarray(Ur_b) + np.asarray(Wsr_b)),
        blk(Uh_w, np.asarray(Uh_b) + np.asarray(Wsh_b)),
    ], axis=1).astype(bf16)
    w2 = np.concatenate([Wsz_w, Wsg_w, Wsr_w, Wsh_w], axis=1).astype(bf16)
    wf = np.asarray(Wf_w, dtype=f32).reshape(HID, 1).astype(bf16)
    return xt_full, w1, w2, wf


def kernel(x, t, Sw_w, Sw_b, Uz_w, Uz_b, Wsz_w, Wsz_b, Ug_w, Ug_b, Wsg_w,
           Wsg_b, Ur_w, Ur_b, Wsr_w, Wsr_b, Uh_w, Uh_b, Wsh_w, Wsh_b, Wf_w,
           Wf_b, n_layers):
    from concourse.bass_utils import run_bass_kernel_spmd

    x = np.asarray(x)
    t = np.asarray(t)
    b_total = x.shape[0]
    assert b_total % N_CORES == 0
    bs = b_total // N_CORES
    n_steps = int(n_layers) - 1

    xt_full, w1, w2, wf = _pack_host(
        x, t, Sw_w, Sw_b, Uz_w, Uz_b, Wsz_w, Wsz_b, Ug_w, Ug_b, Wsg_w, Wsg_b,
        Ur_w, Ur_b, Wsr_w, Wsr_b, Uh_w, Uh_b, Wsh_w, Wsh_b, Wf_w)

    nc = _get_nc(n_steps, bs)

    in_maps = []
    for c in range(N_CORES):
        in_maps.append({
            "xt": np.ascontiguousarray(xt_full[:, c * bs:(c + 1) * bs]),
            "w1": w1,
            "w2": w2,
            "wf": wf,
        })

    res = None
    for attempt in range(3):
        try:
            res = run_bass_kernel_spmd(nc, in_maps,
                                       core_ids=list(range(N_CORES)))
            break
        except Exception:
            if attempt == 2:
                raise
            import time as _time
            _time.sleep(5.0)
    out = np.empty((b_total, 1), dtype=np.float32)
    bf = np.float32(np.asarray(Wf_b).reshape(-1)[0])
    for c in range(N_CORES):
        out[c * bs:(c + 1) * bs, 0] = res.results[c]["out"][0] + bf
    return out



# revision 4
# speedup vs baseline: 1.2192x; 1.0986x over previous
"""DGMNet forward kernel v2 for Trainium2, 8-core data parallel.

Same dataflow as v1 (feature-major activations, bias rows folded into
the matmuls, xt projections recomputed per step as PSUM-accumulating
matmul pairs) but in bf16 end-to-end:

- xt / w1 / w2 / wf live in DRAM+SBUF as bf16 (halves HBM traffic and
  makes every DVE tensor_tensor run in 2x_1P packed mode).
- tanh outputs, S, SR, and all elementwise intermediates are bf16;
  PSUM accumulation stays fp32.
- Engine placement tuned against the timeline cost model and HW
  measurement: ALL elementwise on the DVE (SR, ZS, T1, Snew, output
  copy); the GPSIMD/Pool engine is left idle on purpose -- it shares
  an SBUF port with the DVE on real HW, and moving S*R off it
  measured 488 -> 448 us even though per-op it looked free.
- Final projections are col-tiled (tile_position): 3 tiles' output
  rows land at partitions 0/32/64 of one PSUM bank, so one DVE copy
  and bank evacuate 3 tiles (the copy cost is free-dim driven), and
  the 3 matmuls run concurrently in the PE array's column groups.
- The next group's S1 phase (x DMA + projection + tanh) is interleaved
  into the current group's LAST step, tile by tile, so each S1 reuses
  a PSUM slot freed by an early step-3 tanh and the group boundary has
  no serial S1 pipeline.
"""

import warnings

warnings.filterwarnings("ignore")

import numpy as np

B = 262144
XD = 100
HID = 128
N_CORES = 8
BS = B // N_CORES  # 32768 rows per core
KXT = XD + 2  # x features + t row + ones row
NT = 512  # batch columns per tile


def _build(n_steps: int, bs: int, nt: int, reps: int = 1, hw_loop: int = 0,
           out_copy_engine: str = "vector"):
    import concourse.bacc as bacc
    import concourse.tile as tile
    import concourse.mybir as mybir
    from contextlib import ExitStack, nullcontext

    f32 = mybir.dt.float32
    bf16 = mybir.dt.bfloat16
    Tanh = mybir.ActivationFunctionType.Tanh
    sub = mybir.AluOpType.subtract
    mult = mybir.AluOpType.mult

    ntiles = bs // nt
    nc = bacc.Bacc("TRN2", target_bir_lowering=False, debug=False,
                   num_devices=N_CORES)

    xt = nc.dram_tensor("xt", [KXT, bs], bf16, kind="ExternalInput").ap()
    w1 = nc.dram_tensor("w1", [KXT, 5 * HID], bf16, kind="ExternalInput").ap()
    w2 = nc.dram_tensor("w2", [HID, 4 * HID], bf16, kind="ExternalInput").ap()
    wf = nc.dram_tensor("wf", [HID, 1], bf16, kind="ExternalInput").ap()
    out = nc.dram_tensor("out", [1, bs], f32, kind="ExternalOutput").ap()

    with tile.TileContext(nc) as tc:
        with ExitStack() as ctx:
            consts = ctx.enter_context(tc.tile_pool(name="consts", bufs=1))
            xpool = ctx.enter_context(tc.tile_pool(name="xp", bufs=7))
            spool = ctx.enter_context(tc.tile_pool(name="sp", bufs=12))
            zpool = ctx.enter_context(tc.tile_pool(name="zp", bufs=4))
            epool = ctx.enter_context(tc.tile_pool(name="ep", bufs=6))
            psum = ctx.enter_context(
                tc.tile_pool(name="ps", bufs=2, space="PSUM"))

            w1_t = consts.tile([KXT, 5 * HID], bf16)
            nc.sync.dma_start(w1_t[:], w1[:, :])
            w2_t = consts.tile([HID, 4 * HID], bf16)
            nc.sync.dma_start(w2_t[:], w2[:, :])
            wf_t = consts.tile([HID, 1], bf16)
            nc.sync.dma_start(wf_t[:], wf[:, :])

            def w1s(k):
                return w1_t[:, k * HID:(k + 1) * HID]

            def w2s(k):
                return w2_t[:, k * HID:(k + 1) * HID]

            # Optional HW loop repeating the whole pass (timing rig only)
            loop_cm = (tc.For_i(0, hw_loop, 1,
                                hint_engines=(mybir.EngineType.PE,
                                              mybir.EngineType.Activation,
                                              mybir.EngineType.DVE,
                                              mybir.EngineType.SP,
                                              mybir.EngineType.Pool))
                       if hw_loop else nullcontext())
            ctx.enter_context(loop_cm)

            GRP = 6  # tiles software-pipelined per step loop

            def s1_one(j, i):
                x_t = xpool.tile([KXT, nt], bf16, tag="x", name="x_t")
                nc.sync.dma_start(x_t[:], xt[:, j * nt:(j + 1) * nt])
                ps1 = psum.tile([HID, nt], f32,
                                tag=("r" if i % 2 == 0 else "h"),
                                name="ps1")
                nc.tensor.matmul(ps1[:], w1s(0), x_t[:],
                                 start=True, stop=True)
                S1 = spool.tile([HID, nt], bf16, tag="S", name="S1")
                nc.scalar.activation(S1[:], ps1[:], Tanh)
                return (x_t, S1)

            carry = None
            for j0 in range(0, ntiles * reps, GRP):
                js = [jj % ntiles for jj in
                      range(j0, min(j0 + GRP, ntiles * reps))]
                if carry is None:
                    carry = {}
                    for i, j in enumerate(js):
                        carry[j] = s1_one(j, i)
                xr, S = {}, {}
                for j in js:
                    x_t, S1 = carry[j]
                    xr[j] = x_t[:]
                    S[j] = S1
                nxt = []
                if j0 + GRP < ntiles * reps:
                    nxt = [jj % ntiles for jj in
                           range(j0 + GRP,
                                 min(j0 + 2 * GRP, ntiles * reps))]
                carry = {}

                for _step in range(n_steps):
                    for _idx, j in enumerate(js):
                        Sr = S[j][:]
                        # R first: it unblocks the SR -> H chain
                        pr = psum.tile([HID, nt], f32, tag="r", name="pr")
                        nc.tensor.matmul(pr[:], w1s(3), xr[j],
                                         start=True, stop=False)
                        nc.tensor.matmul(pr[:], w2s(2), Sr,
                                         start=False, stop=True)
                        Rt = zpool.tile([HID, nt], bf16, tag="R", name="Rt")
                        nc.scalar.activation(Rt[:], pr[:], Tanh)
                        SR = epool.tile([HID, nt], bf16, tag="SR", name="SR")
                        nc.vector.tensor_mul(SR[:], S[j][:], Rt[:])

                        pzg = psum.tile([HID, 2 * nt], f32, tag="zg",
                                        name="pzg")
                        for k in range(2):
                            sl = pzg[:, k * nt:(k + 1) * nt]
                            nc.tensor.matmul(sl, w1s(1 + k), xr[j],
                                             start=True, stop=False)
                            nc.tensor.matmul(sl, w2s(k), Sr,
                                             start=False, stop=True)
                        ZG = zpool.tile([HID, 2 * nt], bf16, tag="ZG",
                                        name="ZG")
                        nc.scalar.activation(ZG[:], pzg[:], Tanh)
                        Z = ZG[:, 0:nt]
                        G = ZG[:, nt:2 * nt]

                        ph = psum.tile([HID, nt], f32, tag="h", name="ph")
                        nc.tensor.matmul(ph[:], w1s(4), xr[j],
                                         start=True, stop=False)
                        nc.tensor.matmul(ph[:], w2s(3), SR[:],
                                         start=False, stop=True)

                        ZS = epool.tile([HID, nt], bf16, tag="ZS", name="ZS")
                        nc.vector.tensor_mul(ZS[:], Z, S[j][:])
                        T1 = epool.tile([HID, nt], bf16, tag="T1", name="T1")
                        nc.vector.scalar_tensor_tensor(T1[:], G, 1.0, ph[:],
                                                       op0=sub, op1=mult)
                        Snew = spool.tile([HID, nt], bf16, tag="S",
                                          name="Snew")
                        nc.vector.tensor_sub(Snew[:], ZS[:], T1[:])
                        S[j] = Snew
                        # interleave the NEXT group's S1 phase into this
                        # group's last step: each s1 reuses a psum slot
                        # freed by an EARLY step-3 tanh instead of the
                        # boundary waiting for the last one.
                        if _step == n_steps - 1 and _idx >= 1:
                            k = _idx - 1
                            if k < len(nxt):
                                carry[nxt[k]] = s1_one(nxt[k], k)

                for k in range(max(len(js) - 1, 0), len(nxt)):
                    carry[nxt[k]] = s1_one(nxt[k], k)

                # Final projections, 3 tiles per PSUM bank: col-tiled
                # matmuls land each tile's output row at partition 0/32/64
                # of ONE bank, so a single DVE copy (cost is free-dim
                # driven, partition-count free) evacuates up to 3 tiles.
                for c0 in range(0, len(js), 3):
                    chunk = js[c0:c0 + 3]
                    po = psum.tile([HID, nt], f32, tag="h", name="po")
                    for i, j in enumerate(chunk):
                        nc.tensor.matmul(po[32 * i:32 * i + 1, :], wf_t[:],
                                         S[j][:], start=True, stop=True,
                                         tile_position=(0, 32 * i))
                    o_t = xpool.tile([65, nt], f32, tag="o", name="o_t")
                    nc.vector.tensor_copy(o_t[:], po[0:65, :])
                    for i, j in enumerate(chunk):
                        nc.sync.dma_start(out[:, j * nt:(j + 1) * nt],
                                          o_t[32 * i:32 * i + 1, :])

    nc.compile()
    return nc


_cache = {}


def _get_nc(n_steps: int, bs: int = BS, nt: int = NT):
    key = (n_steps, bs, nt)
    if key not in _cache:
        _cache[key] = _build(n_steps, bs, nt)
    return _cache[key]


def _pack_host(x, t, Sw_w, Sw_b, Uz_w, Uz_b, Wsz_w, Wsz_b, Ug_w, Ug_b, Wsg_w,
               Wsg_b, Ur_w, Ur_b, Wsr_w, Wsr_b, Uh_w, Uh_b, Wsh_w, Wsh_b,
               Wf_w):
    import ml_dtypes

    bf16 = ml_dtypes.bfloat16
    f32 = np.float32
    b_total = x.shape[0]
    xt_full = np.empty((KXT, b_total), dtype=bf16)
    xt_full[:XD, :] = np.asarray(x, dtype=f32).T.astype(bf16)
    xt_full[XD, :] = np.asarray(t, dtype=f32)[:, 0].astype(bf16)
    xt_full[XD + 1, :] = 1.0

    def blk(w, b):
        # [101, 128] weights + folded bias row -> [102, 128]
        return np.concatenate(
            [np.asarray(w, f32),
             np.asarray(b, f32).reshape(1, HID)], axis=0)

    w1 = np.concatenate([
        blk(Sw_w, Sw_b),
        blk(Uz_w, np.asarray(Uz_b) + np.asarray(Wsz_b)),
        blk(Ug_w, np.asarray(Ug_b) + np.asarray(Wsg_b)),
        blk(Ur_w, np.asarray(Ur_b) + np.asarray(Wsr_b)),
        blk(Uh_w, np.asarray(Uh_b) + np.asarray(Wsh_b)),
    ], axis=1).astype(bf16)
    w2 = np.concatenate([Wsz_w, Wsg_w, Wsr_w, Wsh_w], axis=1).astype(bf16)
    wf = np.asarray(Wf_w, dtype=f32).reshape(HID, 1).astype(bf16)
    return xt_full, w1, w2, wf


def kernel(x, t, Sw_w, Sw_b, Uz_w, Uz_b, Wsz_w, Wsz_b, Ug_w, Ug_b, Wsg_w,
           Wsg_b, Ur_w, Ur_b, Wsr_w, Wsr_b, Uh_w, Uh_b, Wsh_w, Wsh_b, Wf_w,
           Wf_b, n_layers):
    from concourse.bass_utils import run_bass_kernel_spmd

    x = np.asarray(x)
    t = np.asarray(t)
    b_total = x.shape[0]
    assert b_total % N_CORES == 0
    bs = b_total // N_CORES
    n_steps = int(n_layers) - 1

    xt_full, w1, w2, wf = _pack_host(
        x, t, Sw_w, Sw_b, Uz_w, Uz_b, Wsz_w, Wsz_b, Ug_w, Ug_b, Wsg_w, Wsg_b,
        Ur_w, Ur_b, Wsr_w, Wsr_b, Uh_w, Uh_b, Wsh_w, Wsh_b, Wf_w)

    nc = _get_nc(n_steps, bs)

    in_maps = []
    for c in range(N_CORES):
        in_maps.append({
            "xt": np.ascontiguousarray(xt_full[:, c * bs:(c + 1) * bs]),
            "w1": w1,
            "w2": w2,
            "wf": wf,
        })

    res = None
    for attempt in range(3):
        try:
            res = run_bass_kernel_spmd(nc, in_maps,
                                       core_ids=list(range(N_CORES)))
            break
        except Exception:
            if attempt == 2:
                raise
            import time as _time
            _time.sleep(5.0)
    out = np.empty((b_total, 1), dtype=np.float32)
    bf = np.float32(np.asarray(Wf_b).reshape(-1)[0])
    for c in range(N_CORES):
        out[c * bs:(c + 1) * bs, 0] = res.results[c]["out"][0] + bf
    return out

